# revision 7
# baseline (speedup 1.0000x reference)
"""DiceLoss (19-class histogram binning) on 8 trn2 NeuronCores.

Strategy (data-parallel over batch):
  - Shard both label tensors over batch across the 8 cores (2 batches each).
  - On each core, compute per-class counts of the local shard:
      hist_y[c], hist_p[c], inter[c] = #(y==c AND y_pred==c) = #(w == 33c)
    for w = 32*y + y_pred (fp16, exact for values <= 594).
  - Counts are fused compare+accumulate passes split across three engines:
      * DVE  : tensor_scalar(is_equal, accum_out=...)  on bf16/fp16 (4x mode)
      * ACT  : activation(Sign, bias=-(c-0.5), accum_out=...) -> exact +/-1
               sums = cumulative counts; histogram = diff of cumulatives.
      * GPSIMD: same tensor_scalar(is_equal) (slower, but a third engine)
    Marginal histograms need 18 passes each (class 0 derived from N);
    the intersection needs 19 equality passes. 55 class-passes total.
  - Per-core output is a small (128, NCHUNK*64) fp32 accumulator tile; host
    sums partitions/chunks/cores (exact integer sums) and evaluates the dice
    formula in float32 to match the jax reference.
"""

import sys

sys.path.insert(0, "/opt/trn_rl_repo")

from contextlib import ExitStack

import numpy as np

import concourse.bass as bass
import concourse.tile as tile
from concourse import bacc, mybir
from concourse.bass_utils import run_bass_kernel_spmd

NCORES = 8
NUM_CLASSES = 19
EPSILON = 1e-05

FULL_SHAPE = (16, 1024, 1024)
P = 128
COLS = (FULL_SHAPE[0] // NCORES) * FULL_SHAPE[1] * FULL_SHAPE[2] // P  # 16384
NCHUNK = 4
ACC_STRIDE = 64

# Engine assignment for the marginal histograms of y and p.
# Classes 1..KCUM-1 are ACT cumulative-sign passes; classes KCUM..18 are
# equality passes, split between GPSIMD (GSET) and DVE (the rest).
KCUM_Y = 8          # y: ACT does cum classes 1..7
KCUM_P = 8          # p: ACT does cum classes 1..7
GSET_Y = ()         # GPSIMD cannot run TensorScalarPtr (walrus engine check)
GSET_P = ()

_CACHE = {}


def _slot_layout():
    """Map each logical quantity to an acc column slot (per chunk).

    Returns (slots, n_used):
      slots["y_cum"][c], slots["y_eq"][c], same for p, slots["w_eq"][c]
    """
    slots = {"y_cum": {}, "y_eq": {}, "p_cum": {}, "p_eq": {}, "w_eq": {}}
    i = 0
    for c in range(1, KCUM_Y):
        slots["y_cum"][c] = i; i += 1
    for c in range(KCUM_Y, NUM_CLASSES):
        slots["y_eq"][c] = i; i += 1
    for c in range(1, KCUM_P):
        slots["p_cum"][c] = i; i += 1
    for c in range(KCUM_P, NUM_CLASSES):
        slots["p_eq"][c] = i; i += 1
    for c in range(NUM_CLASSES):
        slots["w_eq"][c] = i; i += 1
    assert i <= ACC_STRIDE
    return slots, i


def _build_program(cols, nchunk, reps=1, use_gpsimd=False):
    assert cols % nchunk == 0
    F = cols // nchunk
    dt = mybir.dt
    alu = mybir.AluOpType
    afn = mybir.ActivationFunctionType
    eq = alu.is_equal
    slots, _ = _slot_layout()

    nc = bacc.Bacc("TRN2", target_bir_lowering=False, debug=False, num_devices=NCORES)
    y_ap = nc.dram_tensor("y", [P, cols], dt.int32, kind="ExternalInput").ap()
    p_ap = nc.dram_tensor("p", [P, cols], dt.int32, kind="ExternalInput").ap()
    acc_ap = nc.dram_tensor(
        "acc", [P, nchunk * ACC_STRIDE], dt.float32, kind="ExternalOutput"
    ).ap()

    with tile.TileContext(nc) as tc, ExitStack() as ctx:
        in_pool = ctx.enter_context(tc.tile_pool(name="in", bufs=2))
        bf_pool = ctx.enter_context(tc.tile_pool(name="bf", bufs=2))
        trash_pool = ctx.enter_context(tc.tile_pool(name="trash", bufs=2))
        const_pool = ctx.enter_context(tc.tile_pool(name="const", bufs=1))
        acc_pool = ctx.enter_context(tc.tile_pool(name="acc", bufs=1))

        acc = acc_pool.tile([P, nchunk * ACC_STRIDE], dt.float32)
        nc.vector.memset(acc[:], 0.0)

        # Sign biases: one column per cum class of y then p.
        n_bias = (KCUM_Y - 1) + (KCUM_P - 1)
        bias = const_pool.tile([P, max(n_bias, 1)], dt.float32)
        bcol = {}
        bi = 0
        for c in range(1, KCUM_Y):
            nc.vector.memset(bias[:, bi : bi + 1], -(c - 0.5))
            bcol[("y", c)] = bi; bi += 1
        for c in range(1, KCUM_P):
            nc.vector.memset(bias[:, bi : bi + 1], -(c - 0.5))
            bcol[("p", c)] = bi; bi += 1

        def chunk_body(k):
            base = k * ACC_STRIDE

            def aslot(name, c):
                j = base + slots[name][c]
                return acc[:, j : j + 1]

            yi = in_pool.tile([P, F], dt.int32, tag="yi")
            nc.sync.dma_start(yi[:], y_ap[:, k * F : (k + 1) * F])
            pi = in_pool.tile([P, F], dt.int32, tag="pi")
            nc.sync.dma_start(pi[:], p_ap[:, k * F : (k + 1) * F])

            yb = bf_pool.tile([P, F], dt.bfloat16, tag="yb")
            nc.vector.tensor_copy(yb[:], yi[:])
            pb = bf_pool.tile([P, F], dt.bfloat16, tag="pb")
            nc.vector.tensor_copy(pb[:], pi[:])

            # w = 32*y + p (fp16 exact)
            w = bf_pool.tile([P, F], dt.float16, tag="w")
            nc.vector.scalar_tensor_tensor(w[:], yi[:], 32, pi[:], alu.mult, alu.add)

            # --- DVE equality passes ---
            for c in range(NUM_CLASSES):
                t = trash_pool.tile([P, F], dt.float16, tag="t")
                nc.vector.tensor_scalar(
                    t[:], w[:], float(33 * c), None, eq, alu.add,
                    accum_out=aslot("w_eq", c),
                )
            for c in range(KCUM_Y, NUM_CLASSES):
                if use_gpsimd and c in GSET_Y:
                    continue
                t = trash_pool.tile([P, F], dt.bfloat16, tag="t")
                nc.vector.tensor_scalar(
                    t[:], yb[:], float(c), None, eq, alu.add,
                    accum_out=aslot("y_eq", c),
                )
            for c in range(KCUM_P, NUM_CLASSES):
                if use_gpsimd and c in GSET_P:
                    continue
                t = trash_pool.tile([P, F], dt.bfloat16, tag="t")
                nc.vector.tensor_scalar(
                    t[:], pb[:], float(c), None, eq, alu.add,
                    accum_out=aslot("p_eq", c),
                )

            # --- GPSIMD equality passes ---
            if use_gpsimd:
                for c in GSET_Y:
                    t = trash_pool.tile([P, F], dt.bfloat16, tag="tg")
                    nc.gpsimd.tensor_scalar(
                        t[:], yb[:], float(c), None, eq, alu.add,
                        accum_out=aslot("y_eq", c),
                    )
                for c in GSET_P:
                    t = trash_pool.tile([P, F], dt.bfloat16, tag="tg")
                    nc.gpsimd.tensor_scalar(
                        t[:], pb[:], float(c), None, eq, alu.add,
                        accum_out=aslot("p_eq", c),
                    )

            # --- ACT cumulative sign passes ---
            for c in range(1, KCUM_Y):
                t = trash_pool.tile([P, F], dt.bfloat16, tag="ta")
                nc.scalar.activation(
                    t[:], yb[:], afn.Sign, bias=bias[:, bcol[("y", c)] : bcol[("y", c)] + 1],
                    accum_out=aslot("y_cum", c),
                )
            for c in range(1, KCUM_P):
                t = trash_pool.tile([P, F], dt.bfloat16, tag="ta")
                nc.scalar.activation(
                    t[:], pb[:], afn.Sign, bias=bias[:, bcol[("p", c)] : bcol[("p", c)] + 1],
                    accum_out=aslot("p_cum", c),
                )

        if reps == 1:
            for k in range(nchunk):
                chunk_body(k)
        else:
            with tc.For_i(0, reps, 1):
                for k in range(nchunk):
                    chunk_body(k)

        nc.sync.dma_start(acc_ap[:], acc[:])

    nc.compile()
    return nc


def _get_program(cols, nchunk, reps=1, use_gpsimd=False):
    key = (cols, nchunk, reps, use_gpsimd)
    if key not in _CACHE:
        _CACHE[key] = _build_program(cols, nchunk, reps, use_gpsimd)
    return _CACHE[key]


def _decode_marginal(tot, kcum, n_total):
    """tot: dict name->global sums per class slot -> hist (19,) float64."""
    hist = np.zeros(NUM_CLASSES, np.float64)
    # cum[c] = #(x >= c) = (n_total + signsum[c]) / 2, c = 1..kcum-1
    cum = {c: (n_total + tot["cum"][c]) / 2.0 for c in range(1, kcum)}
    for c in range(kcum, NUM_CLASSES):
        hist[c] = tot["eq"][c]
    cum[kcum] = hist[kcum:].sum()
    hist[0] = n_total - cum[1] if kcum > 1 else n_total - hist[1:].sum()
    for c in range(1, kcum):
        hist[c] = cum[c] - cum[c + 1] if (c + 1) in cum else cum[c] - hist[c + 1:].sum()
    return hist


def _histograms_from_acc(accs, n_total):
    slots, _ = _slot_layout()
    g = None
    for a in accs:
        a64 = a.astype(np.float64).reshape(P, -1, ACC_STRIDE).sum(axis=(0, 1))
        g = a64 if g is None else g + a64

    def tot_for(prefix):
        return {
            "cum": {c: g[slots[prefix + "_cum"][c]] for c in slots[prefix + "_cum"]},
            "eq": {c: g[slots[prefix + "_eq"][c]] for c in slots[prefix + "_eq"]},
        }

    hy = _decode_marginal(tot_for("y"), KCUM_Y, n_total)
    hp = _decode_marginal(tot_for("p"), KCUM_P, n_total)
    inter = np.array([g[slots["w_eq"][c]] for c in range(NUM_CLASSES)], np.float64)
    return hy, hp, inter


def _dice_from_counts(count_y, count_p, inter):
    count_y = count_y.astype(np.float32)
    count_p = count_p.astype(np.float32)
    inter = inter.astype(np.float32)
    union = count_y + count_p - inter
    dice = (np.float32(2.0) * inter + np.float32(EPSILON)) / (
        union + np.float32(EPSILON)
    )
    return np.float32(1.0) - np.float32(np.mean(dice))


def kernel(y_pred, y):
    y_pred = np.asarray(y_pred)
    y = np.asarray(y)
    assert y_pred.shape == FULL_SHAPE and y.shape == FULL_SHAPE
    yp32 = np.ascontiguousarray(y_pred, dtype=np.int32)
    y32 = np.ascontiguousarray(y, dtype=np.int32)

    bpc = FULL_SHAPE[0] // NCORES
    in_maps = []
    for i in range(NCORES):
        ysh = y32[i * bpc : (i + 1) * bpc].reshape(P, COLS)
        psh = yp32[i * bpc : (i + 1) * bpc].reshape(P, COLS)
        in_maps.append({"y": ysh, "p": psh})

    nc = _get_program(COLS, NCHUNK)
    res = run_bass_kernel_spmd(nc, in_maps, list(range(NCORES)))
    accs = [res.results[i]["acc"] for i in range(NCORES)]
    n_total = NCORES * P * COLS
    hy, hp, inter = _histograms_from_acc(accs, n_total)
    return _dice_from_counts(hy, hp, inter)


# revision 9
# speedup vs baseline: 1.0895x; 1.0895x over previous
"""DiceLoss (19-class histogram binning) on 8 trn2 NeuronCores.

Strategy (data-parallel over batch, host-side lexicographic pair packing):
  - Shard both label tensors over batch across the 8 cores (2 batches each).
  - The host packs each shard into two int16 tensors:
        w  = (y << 5) | p        w2 = (p << 5) | y      (values <= 594)
    Lexicographic packing makes every needed count a single-source pass:
        inter[c]  = #(w == 33*c)
        cum_y[c]  = #(y >= c) = #(w  >= 32*c - 0.5)   (p < 32)
        cum_p[c]  = #(p >= c) = #(w2 >= 32*c - 0.5)
    and histograms are differences of cumulatives (hist[0] from total N).
    This also halves HBM traffic vs int32 labels (8MB per core).
  - 55 fused compare+accumulate passes per core, split across two engines:
      * DVE: tensor_scalar(is_equal / is_ge, accum_out=...) on int16 tiles
        (single-source 16-bit SBUF ops hit the 4x perf mode).
      * ACT: activation(Sign, bias=-(32c-0.5), accum_out=...) -> exact +/-1
        sums, i.e. cum = (N + signsum) / 2.
  - Per-core output is a small (128, NCHUNK*64) fp32 accumulator tile; host
    sums partitions/chunks/cores (exact integer sums) and evaluates the dice
    formula in float32 to match the jax reference.
"""

import sys

sys.path.insert(0, "/opt/trn_rl_repo")

from contextlib import ExitStack

import numpy as np

import concourse.bass as bass
import concourse.tile as tile
from concourse import bacc, mybir
from concourse.bass_utils import run_bass_kernel_spmd

NCORES = 8
NUM_CLASSES = 19
EPSILON = 1e-05

FULL_SHAPE = (16, 1024, 1024)
P = 128
COLS = (FULL_SHAPE[0] // NCORES) * FULL_SHAPE[1] * FULL_SHAPE[2] // P  # 16384
NCHUNK = 4
ACC_STRIDE = 64

# ACT (scalar engine) takes the cumulative passes for y classes 1..NACT_Y
# and p classes 1..NACT_P; the DVE does the rest (is_ge) plus all 19
# intersection equality passes.
NACT_Y = 7
NACT_P = 6

_CACHE = {}


def _slot_layout():
    """acc column slots per chunk: y_cum[c] 1..18, p_cum[c] 1..18, w_eq[c] 0..18."""
    slots = {"y_cum": {}, "p_cum": {}, "w_eq": {}}
    i = 0
    for c in range(1, NUM_CLASSES):
        slots["y_cum"][c] = i; i += 1
    for c in range(1, NUM_CLASSES):
        slots["p_cum"][c] = i; i += 1
    for c in range(NUM_CLASSES):
        slots["w_eq"][c] = i; i += 1
    assert i <= ACC_STRIDE
    return slots, i


def _build_program(cols, nchunk, reps=1, timing=False):
    assert cols % nchunk == 0
    F = cols // nchunk
    dt = mybir.dt
    alu = mybir.AluOpType
    afn = mybir.ActivationFunctionType
    slots, _ = _slot_layout()

    nc = bacc.Bacc("TRN2", target_bir_lowering=False, debug=False, num_devices=NCORES)
    if timing:
        # Internal DRAM inputs (garbage contents) so timing calls ship no data.
        _ = nc.dram_tensor("tin", [1, 1], dt.int32, kind="ExternalInput").ap()
        w_ap = nc.dram_tensor("w", [P, cols], dt.int16).ap()
        w2_ap = nc.dram_tensor("w2", [P, cols], dt.int16).ap()
    else:
        w_ap = nc.dram_tensor("w", [P, cols], dt.int16, kind="ExternalInput").ap()
        w2_ap = nc.dram_tensor("w2", [P, cols], dt.int16, kind="ExternalInput").ap()
    acc_ap = nc.dram_tensor(
        "acc", [P, nchunk * ACC_STRIDE], dt.float32, kind="ExternalOutput"
    ).ap()

    with tile.TileContext(nc) as tc, ExitStack() as ctx:
        in_pool = ctx.enter_context(tc.tile_pool(name="in", bufs=3))
        trash_pool = ctx.enter_context(tc.tile_pool(name="trash", bufs=2))
        const_pool = ctx.enter_context(tc.tile_pool(name="const", bufs=1))
        acc_pool = ctx.enter_context(tc.tile_pool(name="acc", bufs=1))

        acc = acc_pool.tile([P, nchunk * ACC_STRIDE], dt.float32)
        nc.vector.memset(acc[:], 0.0)

        # Sign biases for ACT cum passes: -(32c - 0.5)
        n_bias = NACT_Y + NACT_P
        bias = const_pool.tile([P, max(n_bias, 1)], dt.float32)
        bcol = {}
        bi = 0
        for c in range(1, NACT_Y + 1):
            nc.vector.memset(bias[:, bi : bi + 1], -(32 * c - 0.5))
            bcol[("y", c)] = bi; bi += 1
        for c in range(1, NACT_P + 1):
            nc.vector.memset(bias[:, bi : bi + 1], -(32 * c - 0.5))
            bcol[("p", c)] = bi; bi += 1

        def chunk_body(k):
            base = k * ACC_STRIDE

            def aslot(name, c):
                j = base + slots[name][c]
                return acc[:, j : j + 1]

            wt = in_pool.tile([P, F], dt.int16, tag="w")
            nc.sync.dma_start(wt[:], w_ap[:, k * F : (k + 1) * F])
            w2t = in_pool.tile([P, F], dt.int16, tag="w2")
            nc.sync.dma_start(w2t[:], w2_ap[:, k * F : (k + 1) * F])

            # --- DVE passes (int16, 4x mode) ---
            for c in range(NUM_CLASSES):
                t = trash_pool.tile([P, F], dt.int16, tag="t")
                nc.vector.tensor_scalar(
                    t[:], wt[:], float(33 * c), None, alu.is_equal, alu.add,
                    accum_out=aslot("w_eq", c),
                )
            for c in range(NACT_Y + 1, NUM_CLASSES):
                t = trash_pool.tile([P, F], dt.int16, tag="t")
                nc.vector.tensor_scalar(
                    t[:], wt[:], 32.0 * c - 0.5, None, alu.is_ge, alu.add,
                    accum_out=aslot("y_cum", c),
                )
            for c in range(NACT_P + 1, NUM_CLASSES):
                t = trash_pool.tile([P, F], dt.int16, tag="t")
                nc.vector.tensor_scalar(
                    t[:], w2t[:], 32.0 * c - 0.5, None, alu.is_ge, alu.add,
                    accum_out=aslot("p_cum", c),
                )

            # --- ACT cumulative sign passes ---
            for c in range(1, NACT_Y + 1):
                t = trash_pool.tile([P, F], dt.bfloat16, tag="ta")
                nc.scalar.activation(
                    t[:], wt[:], afn.Sign,
                    bias=bias[:, bcol[("y", c)] : bcol[("y", c)] + 1],
                    accum_out=aslot("y_cum", c),
                )
            for c in range(1, NACT_P + 1):
                t = trash_pool.tile([P, F], dt.bfloat16, tag="ta")
                nc.scalar.activation(
                    t[:], w2t[:], afn.Sign,
                    bias=bias[:, bcol[("p", c)] : bcol[("p", c)] + 1],
                    accum_out=aslot("p_cum", c),
                )

        if reps == 1:
            for k in range(nchunk):
                chunk_body(k)
        else:
            with tc.For_i(0, reps, 1):
                for k in range(nchunk):
                    chunk_body(k)

        nc.sync.dma_start(acc_ap[:], acc[:])

    nc.compile()
    return nc


def _get_program(cols, nchunk, reps=1, timing=False):
    key = (cols, nchunk, reps, timing)
    if key not in _CACHE:
        _CACHE[key] = _build_program(cols, nchunk, reps, timing)
    return _CACHE[key]


def _histograms_from_acc(accs, n_total):
    """accs: list of (128, nchunk*ACC_STRIDE) fp32 -> (hist_y, hist_p, inter)."""
    slots, _ = _slot_layout()
    g = None
    for a in accs:
        a64 = a.astype(np.float64).reshape(P, -1, ACC_STRIDE).sum(axis=(0, 1))
        g = a64 if g is None else g + a64

    def marginal(prefix, nact):
        cum = np.zeros(NUM_CLASSES + 1, np.float64)
        cum[0] = n_total
        for c in range(1, NUM_CLASSES):
            v = g[slots[prefix + "_cum"][c]]
            # ACT classes hold signsums: cum = (N + s)/2; DVE classes hold counts.
            cum[c] = (n_total + v) / 2.0 if c <= nact else v
        hist = cum[:NUM_CLASSES] - cum[1 : NUM_CLASSES + 1]
        return hist

    hy = marginal("y", NACT_Y)
    hp = marginal("p", NACT_P)
    inter = np.array([g[slots["w_eq"][c]] for c in range(NUM_CLASSES)], np.float64)
    return hy, hp, inter


def _dice_from_counts(count_y, count_p, inter):
    count_y = count_y.astype(np.float32)
    count_p = count_p.astype(np.float32)
    inter = inter.astype(np.float32)
    union = count_y + count_p - inter
    dice = (np.float32(2.0) * inter + np.float32(EPSILON)) / (
        union + np.float32(EPSILON)
    )
    return np.float32(1.0) - np.float32(np.mean(dice))


def _pack_shards(y_pred, y):
    """Full int arrays -> per-core in_maps with packed int16 w/w2."""
    y32 = np.asarray(y).astype(np.int32, copy=False)
    p32 = np.asarray(y_pred).astype(np.int32, copy=False)
    w = ((y32 << 5) | p32).astype(np.int16).reshape(NCORES, P, COLS)
    w2 = ((p32 << 5) | y32).astype(np.int16).reshape(NCORES, P, COLS)
    return [{"w": w[i], "w2": w2[i]} for i in range(NCORES)]


def kernel(y_pred, y):
    y_pred = np.asarray(y_pred)
    y = np.asarray(y)
    assert y_pred.shape == FULL_SHAPE and y.shape == FULL_SHAPE
    in_maps = _pack_shards(y_pred, y)
    nc = _get_program(COLS, NCHUNK)
    res = run_bass_kernel_spmd(nc, in_maps, list(range(NCORES)))
    accs = [res.results[i]["acc"] for i in range(NCORES)]
    n_total = NCORES * P * COLS
    hy, hp, inter = _histograms_from_acc(accs, n_total)
    return _dice_from_counts(hy, hp, inter)
